# revision 1
# baseline (speedup 1.0000x reference)
"""Trainium2 Bass kernel for fp8 quantize-dequantize DenseGeneral + gelu.

Computes: out = gelu(qdq_e4m3fn(x) @ qdq_e4m3fn(W) + round_bf16(bias))
with delayed-scaling fp8 quantization (scale = amax/448 over full tensor,
folded with the amax history), reproducing reference.py numerics.

Distribution (8 NeuronCores, tensor-parallel on F):
  - x [8192, 4096] is token-sharded for the quantize phase (1024 tokens/core);
    each core computes a local abs-max, quantizes+transposes its shard to
    fp8, then an AllGather replicates the full quantized x^T to all cores.
  - W [4096, 16384] and bias are column-sharded (2048 cols/core). Local
    abs-max; quantized W shard stays SBUF-resident.
  - A single 2-float AllReduce(max) produces the global amaxes.
  - Each core computes out[:, its 2048 cols] and the host concatenates.

fp8 trick: TRN's float8e4 has max 240 (OCP e4m3fn has 448). We store q/2:
multiplying by a power of two preserves round-to-nearest decisions on the
3-bit-mantissa grid, so RNE(v/2) in TRN-fp8 == RNE(v)/2 in e4m3fn for all
|v| >= 2^-5 (below that, absolute error <= 2^-9 * scale - negligible).
The factor 4 is folded into the output scale C = 4 * s_x * s_w.
The matmul runs in fp8 DoubleRow mode (2 fp8 MACs/cell/cycle).
"""

import sys

sys.path.insert(0, "/opt/trn_rl_repo")

import numpy as np
from contextlib import ExitStack

import concourse.bass as bass
import concourse.mybir as mybir
import concourse.tile as tile
from concourse.tile import add_dep_helper
from concourse import bacc, bass_isa
from concourse.bass_utils import run_bass_kernel_spmd
from concourse.bass_interp import get_hw_module
from concourse.masks import make_identity

F32 = mybir.dt.float32
BF16 = mybir.dt.bfloat16
FP8 = mybir.dt.float8e4
AX = mybir.AxisListType
ALU = mybir.AluOpType
ACTF = mybir.ActivationFunctionType
DR = mybir.MatmulPerfMode.DoubleRow

# Problem shapes (hardcoded per contract)
B, S, D, F = 4, 2048, 4096, 16384
T = B * S
NCORES = 8
HL = 16
E4M3_MAX = 448.0


def build_program(t_shard, d, f_shard, n_cores, hl=HL, act_fn=ACTF.Gelu_apprx_tanh):
    """Build the SPMD per-core bass program. Same program on every core;
    per-core behavior differs only through the input shards."""
    P = 128
    t_total = t_shard * n_cores
    t_tiles = t_shard // P          # token tiles in this core's x shard
    d_tiles = d // P                # contraction subtiles
    NF = 512                        # psum free dim
    n_tiles = f_shard // NF
    d_half = min(d, 1024)           # x amax/quant DMA chunk (free dim)
    d_chunks = d // d_half
    MSUP = min(512, t_shard // 2)   # tokens per streamed lhsT tile
    assert d_tiles % 2 == 0, "DoubleRow needs an even number of k-subtiles"

    nc = bacc.Bacc(
        "TRN2",
        target_bir_lowering=False,
        debug=False,
        num_devices=n_cores,
    )

    x_sh = nc.dram_tensor("x_shard", [t_shard, d], F32, kind="ExternalInput")
    w_sh = nc.dram_tensor("w_shard", [d, f_shard], F32, kind="ExternalInput")
    b_sh = nc.dram_tensor("bias_shard", [1, f_shard], F32, kind="ExternalInput")
    ih = nc.dram_tensor("in_hist", [1, hl], F32, kind="ExternalInput")
    kh = nc.dram_tensor("k_hist", [1, hl], F32, kind="ExternalInput")
    out_sh = nc.dram_tensor("out_shard", [t_total, f_shard], F32, kind="ExternalOutput")

    rg = [list(range(n_cores))]
    shared = "Shared" if n_cores > 4 else "Local"

    with tile.TileContext(nc) as tc, ExitStack() as ctx:
        const = ctx.enter_context(tc.tile_pool(name="const", bufs=1))
        small = ctx.enter_context(tc.tile_pool(name="small", bufs=1))
        xs = ctx.enter_context(tc.tile_pool(name="xs", bufs=2))
        ws = ctx.enter_context(tc.tile_pool(name="ws", bufs=2))
        qwp = ctx.enter_context(tc.tile_pool(name="qw", bufs=1))
        qxs = ctx.enter_context(tc.tile_pool(name="qxs", bufs=1))
        lhsp = ctx.enter_context(tc.tile_pool(name="lhs", bufs=2))
        stg = ctx.enter_context(tc.tile_pool(name="stg", bufs=3))
        psum = ctx.enter_context(tc.tile_pool(name="psum", bufs=8, space="PSUM"))
        dram = ctx.enter_context(tc.tile_pool(name="dram", bufs=1, space="DRAM"))

        # ---- constants ----
        ident = const.tile([P, P], F32)
        make_identity(nc, ident)
        zbias = const.tile([P, 1], F32)
        nc.gpsimd.memset(zbias[:], 0.0)

        # ---- phase 1: local abs-max of x shard and w shard ----
        histx = small.tile([1, hl], F32)
        nc.sync.dma_start(histx[:], ih[:])
        histw = small.tile([1, hl], F32)
        nc.sync.dma_start(histw[:], kh[:])

        # x stream first: its amax -> AR -> quant -> AllGather is the critical
        # chain (the AG costs ~150us on its own); w's chain hides behind it.
        n_xc = t_tiles * d_chunks
        xacc = small.tile([P, n_xc], F32)
        wacc = small.tile([P, d_tiles], F32)
        last_x1_dma = None
        for step in range(n_xc):
            i, h = divmod(step, d_chunks)
            xt = xs.tile([P, d_half], F32)
            last_x1_dma = nc.sync.dma_start(
                xt[:], x_sh[i * P : (i + 1) * P, h * d_half : (h + 1) * d_half]
            )
            nc.vector.reduce_max(
                xacc[:, step : step + 1], xt[:], axis=AX.X,
                apply_absolute_value=True,
            )
        def armax(acc, sfx):
            mp = small.tile([P, 1], F32, name=f"mp_{sfx}")
            nc.vector.reduce_max(mp[:], acc[:], axis=AX.X)
            ma = small.tile([P, 1], F32, name=f"ma_{sfx}")
            nc.gpsimd.partition_all_reduce(ma[:], mp[:], 128, bass_isa.ReduceOp.max)
            ar_in = dram.tile([1, 1], F32, name=f"arin_{sfx}")
            ar_out = dram.tile([1, 1], F32, addr_space=shared, name=f"arout_{sfx}")
            nc.sync.dma_start(ar_in[:], ma[0:1, :])
            nc.gpsimd.collective_compute(
                "AllReduce",
                ALU.max,
                replica_groups=rg,
                ins=[ar_in[:].opt()],
                outs=[ar_out[:].opt()],
            )
            g = small.tile([1, 1], F32, name=f"g_{sfx}")
            nc.sync.dma_start(g[:], ar_out[:])
            return g

        # x's AR is emitted before any w work so its final DVE reduce isn't
        # queued behind the 32 w reduces; w's AR only gates the w-quant
        # stream that the matmul k-loop chases.
        gmax_x = armax(xacc, "x")

        for step in range(d_tiles):
            wt = ws.tile([P, f_shard], F32)
            nc.scalar.dma_start(wt[:], w_sh[step * P : (step + 1) * P, :])
            nc.vector.reduce_max(
                wacc[:, step : step + 1], wt[:], axis=AX.X,
                apply_absolute_value=True,
            )
        gmax_w = armax(wacc, "w")

        # ---- phase 2: scales ----
        # reference: hist' = [amax_now, hist[0:HL-1]]; amax = max(hist')
        #            sf = 448/amax ; s = 1/sf (dequant scale)
        # ours:      r_half = 0.5*sf (quant multiplier, half-scale trick)
        #            C = 4 * s_x * s_w (output scale)
        def scales(gm, hist, sfx):
            hmx = small.tile([1, 1], F32, name=f"hmx_{sfx}")
            nc.vector.reduce_max(hmx[:], hist[:, 0 : hl - 1], axis=AX.X)
            amax = small.tile([1, 1], F32, name=f"amax_{sfx}")
            nc.vector.tensor_tensor(amax[:], gm, hmx[:], op=ALU.max)
            ra = small.tile([1, 1], F32, name=f"ra_{sfx}")
            nc.vector.reciprocal(ra[:], amax[:])
            sf = small.tile([1, 1], F32, name=f"sf_{sfx}")
            nc.vector.tensor_scalar_mul(sf[:], ra[:], E4M3_MAX)
            s = small.tile([1, 1], F32, name=f"s_{sfx}")
            nc.vector.reciprocal(s[:], sf[:])
            rh = small.tile([1, 1], F32, name=f"rh_{sfx}")
            nc.vector.tensor_scalar_mul(rh[:], sf[:], 0.5)
            return s, rh

        s_x, rh_x = scales(gmax_x[:], histx, "x")
        s_w, rh_w = scales(gmax_w[:], histw, "w")

        Cs = small.tile([1, 1], F32)
        nc.vector.tensor_tensor(Cs[:], s_x[:], s_w[:], op=ALU.mult)
        nc.vector.tensor_scalar_mul(Cs[:], Cs[:], 4.0)
        rC = small.tile([1, 1], F32)
        nc.vector.reciprocal(rC[:], Cs[:])

        rhx_b = small.tile([P, 1], F32)
        nc.gpsimd.partition_broadcast(rhx_b[:], rh_x[:])
        rhw_b = small.tile([P, 1], F32)
        nc.gpsimd.partition_broadcast(rhw_b[:], rh_w[:])
        C_b = small.tile([P, 1], F32)
        nc.gpsimd.partition_broadcast(C_b[:], Cs[:])

        # bias: fp32 -> bf16 -> fp32, then pre-divide by C, broadcast to 128 parts
        btmp = small.tile([1, f_shard], F32)
        nc.sync.dma_start(btmp[:], b_sh[:])
        bbf = small.tile([1, f_shard], BF16)
        nc.vector.tensor_copy(bbf[:], btmp[:])
        nc.vector.tensor_copy(btmp[:], bbf[:])
        nc.vector.tensor_scalar_mul(btmp[:], btmp[:], rC[:])
        bP = small.tile([P, f_shard], F32)
        nc.gpsimd.partition_broadcast(bP[:], btmp[:])

        # ---- phase 3: quantize + transpose x shard -> qxT (fp8) ----
        # The shard is split into two token-halves, each with its own
        # SBUF-resident tile, DRAM staging block, and AllGather. The two AGs
        # pipeline on the collective engine: foreign-block matmuls can start
        # as soon as AG_A lands (full contraction depth - the split is on
        # tokens, not d), and own-block matmuls only wait for their half's
        # quantization, not the whole shard.
        TH = t_shard // 2
        th_tiles = TH // P
        qxt_dram = dram.tile([2, d, TH], FP8)
        qxt_sb_h = [
            qxs.tile([P, d_tiles, TH], FP8, name=f"qxtsb{h}") for h in range(2)
        ]
        qxt_all_h = [
            dram.tile([n_cores * d, TH], FP8, addr_space=shared, name=f"qxtall{h}")
            for h in range(2)
        ]
        for th in range(2):
            for it in range(th_tiles):
                i = th * th_tiles + it
                for h in range(d_chunks):
                    xt = xs.tile([P, d_half], F32)
                    nc.sync.dma_start(
                        xt[:],
                        x_sh[i * P : (i + 1) * P, h * d_half : (h + 1) * d_half],
                    )
                    # pack 4 transposed 128x128 tiles per PSUM bank: 4x the
                    # pre-scale parking capacity, and the quant drains in one
                    # [128, 512] DVE op instead of four [128, 128] ones
                    for g in range(d_half // P // 4):
                        pt = psum.tile([P, NF], F32, tag="ps")
                        for j4 in range(4):
                            j = g * 4 + j4
                            nc.tensor.transpose(
                                pt[:, j4 * P : (j4 + 1) * P],
                                xt[:, j * P : (j + 1) * P],
                                ident[:],
                            )
                        dj = h * (d_half // P) + g * 4
                        # quantize on ScalarE: it's idle during the prologue
                        # (DVE is busy with the amax reduces) and is the
                        # engine closest to PSUM
                        nc.scalar.mul(
                            qxt_sb_h[th][:, dj : dj + 4, it * P : (it + 1) * P],
                            pt[:],
                            rhx_b[:],
                        )
            for dj in range(d_tiles):
                # on the gpsimd queue (same as the AG trigger): fires right
                # behind the quant ops, not behind the sync queue's x streams
                nc.gpsimd.dma_start(
                    qxt_dram[th, dj * P : (dj + 1) * P, :], qxt_sb_h[th][:, dj, :]
                )
            nc.gpsimd.collective_compute(
                "AllGather",
                ALU.bypass,
                replica_groups=rg,
                ins=[qxt_dram[th].opt()],
                outs=[qxt_all_h[th][:].opt()],
            )

        # ---- phase 4: quantize w shard (SBUF resident) ----
        # one tile per DoubleRow pair so the matmul k-loop can stream behind
        # the quantization instead of waiting for the whole [d, f] tensor.
        qw_tiles = [
            qwp.tile([P, 2, f_shard], FP8, name=f"qwt{s}")
            for s in range(d_tiles // 2)
        ]
        for j in range(d_tiles):
            wt = ws.tile([P, f_shard], F32)
            nc.sync.dma_start(wt[:], w_sh[j * P : (j + 1) * P, :])
            nc.scalar.mul(qw_tiles[j // 2][:, j % 2, :], wt[:], rhw_b[:])

        # ---- phase 5: matmul + epilogue ----
        # out[tok, f] = gelu(C * (sum_d qxT[d, tok] * qw[d, f] + bias/C))
        # Block order is rotated per core: slot 0 is the core's OWN token
        # block, read straight from SBUF (qxt_sb) with no AllGather
        # dependency, so the PE starts while the AG is in flight. Slots 1..7
        # read the AG result at a runtime (partition-id based) offset, and
        # every output DMA row offset is runtime-computed to land the slot at
        # its global token position.
        pid_g = nc.gpsimd.partition_id()
        pid_s = nc.sync.partition_id()

        def mm_chunk(lh, mts, mc0, row0):
            # one chunk: mts m-tiles starting at column mc0 of lh, output
            # rows starting at row0 (RuntimeValue)
            for mt in range(mts):
                pss = [
                    psum.tile([P, NF], F32, tag="ps", name=f"mmps{n}")
                    for n in range(n_tiles)
                ]
                mc = mc0 + mt * P
                for ks in range(d_tiles // 2):
                    for n in range(n_tiles):
                        nc.tensor.matmul(
                            pss[n][:],
                            lhsT=lh[:, 2 * ks : 2 * ks + 2, mc : mc + P],
                            rhs=qw_tiles[ks][:, :, n * NF : (n + 1) * NF],
                            start=(ks == 0),
                            stop=(ks == d_tiles // 2 - 1),
                            perf_mode=DR,
                        )
                row = row0 + mt * P
                for n in range(n_tiles):
                    t1 = stg.tile([P, NF], F32)
                    nc.vector.tensor_tensor(
                        t1[:], pss[n][:], bP[:, n * NF : (n + 1) * NF], op=ALU.add
                    )
                    ot = stg.tile([P, NF], F32)
                    nc.scalar.activation(
                        ot[:], t1[:], act_fn, bias=zbias[:], scale=C_b[:]
                    )
                    nc.gpsimd.dma_start(
                        out_sh[bass.ds(row, P), n * NF : (n + 1) * NF], ot[:]
                    )

        # own block first (SBUF, no AG dependency), then all foreign blocks'
        # first token-halves (needs AG_A only), then the second halves - so
        # AG_B has ~7 blocks x 33us of PE work to hide behind.
        row_g0 = pid_g * t_shard
        for th in range(2):
            mm_chunk(qxt_sb_h[th], TH // P, 0, row_g0 + th * TH)
        for th in range(2):
            for slot in range(1, n_cores):
                row_g = ((pid_g + slot) % n_cores) * t_shard
                bi_s = (pid_s + slot) % n_cores
                src = qxt_all_h[th][bass.ds(bi_s * d, d), :].rearrange(
                    "(s p) m -> p s m", p=P, s=d_tiles
                )
                for q in range(TH // MSUP):
                    lh = lhsp.tile([P, d_tiles, MSUP], FP8, name="lh")
                    nc.sync.dma_start(
                        lh[:], src[:, :, q * MSUP : (q + 1) * MSUP]
                    )
                    mm_chunk(lh, MSUP // P, 0, row_g + th * TH + q * MSUP)

    nc.compile()
    return nc


_CACHE = {}


def _get_program(t_shard=T // NCORES, d=D, f_shard=F // NCORES, n_cores=NCORES):
    key = (t_shard, d, f_shard, n_cores)
    if key not in _CACHE:
        _CACHE[key] = build_program(t_shard, d, f_shard, n_cores)
    return _CACHE[key]


def make_in_maps(x, w, bias, in_hist, k_hist, n_cores=NCORES):
    t_shard = x.shape[0] // n_cores
    f_shard = w.shape[1] // n_cores
    in_maps = []
    for r in range(n_cores):
        in_maps.append(
            {
                "x_shard": np.ascontiguousarray(
                    x[r * t_shard : (r + 1) * t_shard], dtype=np.float32
                ),
                "w_shard": np.ascontiguousarray(
                    w[:, r * f_shard : (r + 1) * f_shard], dtype=np.float32
                ),
                "bias_shard": np.ascontiguousarray(
                    bias[r * f_shard : (r + 1) * f_shard], dtype=np.float32
                ).reshape(1, f_shard),
                "in_hist": np.asarray(in_hist, np.float32).reshape(1, HL),
                "k_hist": np.asarray(k_hist, np.float32).reshape(1, HL),
            }
        )
    return in_maps


def _install_ntff_shim():
    """Provide antenv.axon_hooks (absent in this image) so bass_utils can
    NTFF-profile under axon, wiring it to libaxon_pjrt's nrt profile API."""
    import sys as _sys
    import types

    if "antenv.axon_hooks" in _sys.modules:
        return
    mod = types.ModuleType("antenv.axon_hooks")
    _state = {"hook": None}
    mod.set_axon_ntff_profile_hook = lambda h: _state.__setitem__("hook", h)
    mod.get_axon_ntff_profile_hook = lambda: _state["hook"]
    _sys.modules["antenv.axon_hooks"] = mod
    import antenv

    antenv.axon_hooks = mod
    try:
        from trn_agent_boot.trn_boot import _ntff_profile_via_ctypes

        mod.set_axon_ntff_profile_hook(
            _ntff_profile_via_ctypes("/opt/axon/libaxon_pjrt.so")
        )
    except Exception as e:
        print(f"ntff shim: hook unavailable ({e}); tracing will degrade")


def run(inputs_dict, trace=False, **kw):
    """Run on the 8 NeuronCores; returns (full_output, BassKernelResults)."""
    if trace:
        _install_ntff_shim()
    x = np.asarray(inputs_dict["inputs"], np.float32).reshape(T, D)
    w = np.asarray(inputs_dict["kernel"], np.float32)
    bias = np.asarray(inputs_dict["bias"], np.float32)
    nc = _get_program()
    in_maps = make_in_maps(
        x, w, bias, inputs_dict["input_amax_history"], inputs_dict["kernel_amax_history"]
    )
    old_m = nc.m
    nc.m = get_hw_module(nc.m)
    try:
        res = run_bass_kernel_spmd(
            nc, in_maps, core_ids=list(range(NCORES)), trace=trace, **kw
        )
    finally:
        nc.m = old_m
    f_shard = F // NCORES
    out = np.concatenate(
        [res.results[r]["out_shard"] for r in range(NCORES)], axis=1
    )
    return out.reshape(B, S, F).astype(np.float32), res


def kernel(**inputs):
    out, _ = run(inputs, trace=False)
    return out



# revision 5
# speedup vs baseline: 1.2623x; 1.2623x over previous
"""Trainium2 Bass kernel for fp8 quantize-dequantize DenseGeneral + gelu.

Computes: out = gelu(qdq_e4m3fn(x) @ qdq_e4m3fn(W) + round_bf16(bias))
with delayed-scaling fp8 quantization (scale = amax/448 over full tensor,
folded with the amax history), reproducing reference.py numerics.

Distribution (8 NeuronCores, tensor-parallel on F):
  - The host pre-transposes x to x^T and replicates it to every core's
    DRAM in fp16, laid out chunk-major ([64, 128, 32, 128]: 1 MB fully
    contiguous per 128-token chunk) so the matmul lhsT stream runs at
    DMA line rate. Each core computes out[:, its 2048 f-columns] for ALL
    tokens, quantizing x^T chunks on the fly. No AllGather, no PE
    transposes, no runtime-offset DMAs.
  - Exact fp32 amax: each core reduces a disjoint 1/8 token-slice of x^T
    (x_loc, fp32) and its fp32 W column shard; one 2-float
    AllReduce(max) produces both global amaxes.
  - W is quantized from a pair-major fp16 copy (w16) streamed once;
    quant muls alternate ScalarE/VectorE so the matmul k-loop can chase.
  - fp16 transport adds ~0.28% fp8 rounding flips vs fp32 source
    (measured offline: rel err 9.1e-3 vs the 2e-2 gate).

fp8 trick: TRN's float8e4 has max 240 (OCP e4m3fn has 448). We store q/2:
multiplying by a power of two preserves round-to-nearest decisions on the
3-bit-mantissa grid, so RNE(v/2) in TRN-fp8 == RNE(v)/2 in e4m3fn for all
|v| >= 2^-5 (below that, absolute error <= 2^-9 * scale - negligible).
The factor 4 is folded into the output scale C = 4 * s_x * s_w.
The matmul runs in fp8 DoubleRow mode (2 fp8 MACs/cell/cycle).
"""

import sys

sys.path.insert(0, "/opt/trn_rl_repo")

import numpy as np
from contextlib import ExitStack

import concourse.bass as bass
import concourse.mybir as mybir
import concourse.tile as tile
from concourse import bacc, bass_isa
from concourse.bass_utils import run_bass_kernel_spmd
from concourse.bass_interp import get_hw_module

F32 = mybir.dt.float32
F16 = mybir.dt.float16
BF16 = mybir.dt.bfloat16
FP8 = mybir.dt.float8e4
AX = mybir.AxisListType
ALU = mybir.AluOpType
ACTF = mybir.ActivationFunctionType
DR = mybir.MatmulPerfMode.DoubleRow

# Problem shapes (hardcoded per contract)
B, S, D, F = 4, 2048, 4096, 16384
T = B * S
NCORES = 8
HL = 16
E4M3_MAX = 448.0
P = 128
MS = 128                 # tokens per lhsT chunk


def build_program(d, f_shard, t_total, n_cores, hl=HL, act_fn=ACTF.Gelu_apprx_tanh):
    """Build the SPMD per-core bass program. Same program on every core;
    per-core behavior differs only through the input shards."""
    t_loc = t_total // n_cores   # tokens in this core's amax slice
    d_tiles = d // P             # 32 contraction subtiles
    n_pairs = d_tiles // 2       # 16 DoubleRow k-pairs
    NF = 512                     # psum free dim
    n_tiles = f_shard // NF      # 4
    n_chunks = t_total // MS     # 64

    nc = bacc.Bacc(
        "TRN2",
        target_bir_lowering=False,
        debug=False,
        num_devices=n_cores,
    )

    xt16 = nc.dram_tensor("xt16", [n_chunks, P, d_tiles, MS], F16, kind="ExternalInput")
    x_loc = nc.dram_tensor("x_loc", [d, t_loc], F32, kind="ExternalInput")
    w_sh = nc.dram_tensor("w_shard", [d, f_shard], F32, kind="ExternalInput")
    w16 = nc.dram_tensor("w16", [n_pairs, P, 2, f_shard], F16, kind="ExternalInput")
    b_sh = nc.dram_tensor("bias_shard", [1, f_shard], F32, kind="ExternalInput")
    ih = nc.dram_tensor("in_hist", [1, hl], F32, kind="ExternalInput")
    kh = nc.dram_tensor("k_hist", [1, hl], F32, kind="ExternalInput")
    out_sh = nc.dram_tensor("out_shard", [t_total, f_shard], F32, kind="ExternalOutput")

    rg = [list(range(n_cores))]
    shared = "Shared" if n_cores > 4 else "Local"

    with tile.TileContext(nc) as tc, ExitStack() as ctx:
        const = ctx.enter_context(tc.tile_pool(name="const", bufs=1))
        small = ctx.enter_context(tc.tile_pool(name="small", bufs=1))
        xs = ctx.enter_context(tc.tile_pool(name="xs", bufs=3))
        wsa = ctx.enter_context(tc.tile_pool(name="wsa", bufs=3))
        wsr = ctx.enter_context(tc.tile_pool(name="wsr", bufs=3))
        qwp = ctx.enter_context(tc.tile_pool(name="qw", bufs=1))
        lfp = ctx.enter_context(tc.tile_pool(name="lfp", bufs=3))
        qlp = ctx.enter_context(tc.tile_pool(name="qlp", bufs=3))
        ostg = ctx.enter_context(tc.tile_pool(name="ostg", bufs=4))
        psum = ctx.enter_context(tc.tile_pool(name="psum", bufs=8, space="PSUM"))
        dram = ctx.enter_context(tc.tile_pool(name="dram", bufs=1, space="DRAM"))

        # DMA-trigger queues for the bulk prologue streams.
        queues = [nc.sync, nc.scalar, nc.gpsimd]

        # ---- constants ----
        zbias = const.tile([P, 1], F32)
        nc.gpsimd.memset(zbias[:], 0.0)

        histx = small.tile([1, hl], F32)
        nc.gpsimd.dma_start(histx[:], ih[:])
        histw = small.tile([1, hl], F32)
        nc.gpsimd.dma_start(histw[:], kh[:])

        # ---- phase 1: local abs-max of x_loc and w shard (both fp32) ----
        xacc = small.tile([P, d_tiles], F32)
        wacc = small.tile([P, d_tiles], F32)
        qi = 0
        for i in range(d_tiles):
            xt = xs.tile([P, t_loc], F32, name="xt")
            queues[qi % 3].dma_start(xt[:], x_loc[i * P : (i + 1) * P, :])
            qi += 1
            nc.vector.reduce_max(
                xacc[:, i : i + 1], xt[:], axis=AX.X, apply_absolute_value=True
            )
            wt = wsa.tile([P, f_shard], F32, name="wta")
            queues[qi % 3].dma_start(wt[:], w_sh[i * P : (i + 1) * P, :])
            qi += 1
            nc.vector.reduce_max(
                wacc[:, i : i + 1], wt[:], axis=AX.X, apply_absolute_value=True
            )

        # cross-partition reduce both, pack into [1,2], ONE AllReduce(max)
        ar_in = dram.tile([1, 2], F32, name="arin")
        ar_out = dram.tile([1, 2], F32, addr_space=shared, name="arout")

        def pack_amax(acc, col, sfx):
            mp = small.tile([P, 1], F32, name=f"mp_{sfx}")
            nc.vector.reduce_max(mp[:], acc[:], axis=AX.X)
            ma = small.tile([P, 1], F32, name=f"ma_{sfx}")
            nc.gpsimd.partition_all_reduce(ma[:], mp[:], 128, bass_isa.ReduceOp.max)
            nc.gpsimd.dma_start(ar_in[:, col : col + 1], ma[0:1, :])

        pack_amax(xacc, 0, "x")
        pack_amax(wacc, 1, "w")
        nc.gpsimd.collective_compute(
            "AllReduce",
            ALU.max,
            replica_groups=rg,
            ins=[ar_in[:].opt()],
            outs=[ar_out[:].opt()],
        )
        g = small.tile([1, 2], F32)
        nc.gpsimd.dma_start(g[:], ar_out[:])

        # ---- phase 2: scales ----
        # reference: hist' = [amax_now, hist[0:HL-1]]; amax = max(hist')
        #            sf = 448/amax ; s = 1/sf (dequant scale)
        # ours:      r_half = 0.5*sf (quant multiplier, half-scale trick)
        #            C = 4 * s_x * s_w (output scale)
        def scales(gm, hist, sfx):
            hmx = small.tile([1, 1], F32, name=f"hmx_{sfx}")
            nc.vector.reduce_max(hmx[:], hist[:, 0 : hl - 1], axis=AX.X)
            amax = small.tile([1, 1], F32, name=f"amax_{sfx}")
            nc.vector.tensor_tensor(amax[:], gm, hmx[:], op=ALU.max)
            ra = small.tile([1, 1], F32, name=f"ra_{sfx}")
            nc.vector.reciprocal(ra[:], amax[:])
            sf = small.tile([1, 1], F32, name=f"sf_{sfx}")
            nc.vector.tensor_scalar_mul(sf[:], ra[:], E4M3_MAX)
            s = small.tile([1, 1], F32, name=f"s_{sfx}")
            nc.vector.reciprocal(s[:], sf[:])
            rh = small.tile([1, 1], F32, name=f"rh_{sfx}")
            nc.vector.tensor_scalar_mul(rh[:], sf[:], 0.5)
            return s, rh

        s_x, rh_x = scales(g[:, 0:1], histx, "x")
        s_w, rh_w = scales(g[:, 1:2], histw, "w")

        Cs = small.tile([1, 1], F32)
        nc.vector.tensor_tensor(Cs[:], s_x[:], s_w[:], op=ALU.mult)
        nc.vector.tensor_scalar_mul(Cs[:], Cs[:], 4.0)
        rC = small.tile([1, 1], F32)
        nc.vector.reciprocal(rC[:], Cs[:])

        rhx_b = small.tile([P, 1], F32)
        nc.gpsimd.partition_broadcast(rhx_b[:], rh_x[:])
        rhw_b = small.tile([P, 1], F32)
        nc.gpsimd.partition_broadcast(rhw_b[:], rh_w[:])
        C_b = small.tile([P, 1], F32)
        nc.gpsimd.partition_broadcast(C_b[:], Cs[:])

        # bias: fp32 -> bf16 -> fp32, then pre-divide by C, broadcast to 128 parts
        btmp = small.tile([1, f_shard], F32)
        nc.gpsimd.dma_start(btmp[:], b_sh[:])
        bbf = small.tile([1, f_shard], BF16)
        nc.vector.tensor_copy(bbf[:], btmp[:])
        nc.vector.tensor_copy(btmp[:], bbf[:])
        nc.vector.tensor_scalar_mul(btmp[:], btmp[:], rC[:])
        bP = small.tile([P, f_shard], F32)
        nc.gpsimd.partition_broadcast(bP[:], btmp[:])

        # lhsT chunk loader: 1 MB contiguous fp16 DMA + quantize on ScalarE
        def load_chunk(q):
            lf = lfp.tile([P, d_tiles, MS], F16, name="lf")
            nc.sync.dma_start(lf[:], xt16[q])
            ql = qlp.tile([P, d_tiles, MS], FP8, name="ql")
            nc.scalar.mul(ql[:], lf[:], rhx_b[:])
            return ql

        # preload chunks 0/1 ahead of the w-quant muls in the ScalarE FIFO
        # so the first matmuls fire right after the AllReduce
        pre = {q: load_chunk(q) for q in (0, 1)}

        # ---- phase 3: quantize w shard from the fp16 copy (SBUF fp8) ----
        # Quant muls alternate ScalarE/VectorE so the matmul k-loop chases
        # at ~1.5us/pair instead of 3us.
        qw_tiles = [
            qwp.tile([P, 2, f_shard], FP8, name=f"qwt{k}") for k in range(n_pairs)
        ]
        for k in range(n_pairs):
            wt = wsr.tile([P, 2, f_shard], F16, name="wtr")
            queues[(k % 2) + 1].dma_start(wt[:], w16[k])
            if k % 2 == 0:
                nc.scalar.mul(qw_tiles[k][:], wt[:], rhw_b[:])
            else:
                nc.vector.tensor_scalar_mul(qw_tiles[k][:], wt[:], rhw_b[:])

        # ---- phase 4: matmul + epilogue ----
        # out[tok, f] = gelu(C * (sum_d qxT[d, tok] * qw[d, f] + bias/C))
        for q in range(n_chunks):
            ql = pre.pop(q) if q in pre else load_chunk(q)
            pss = [
                psum.tile([P, NF], F32, tag="ps", name=f"mmps{n}")
                for n in range(n_tiles)
            ]
            for k in range(n_pairs):
                for n in range(n_tiles):
                    nc.tensor.matmul(
                        pss[n][:],
                        lhsT=ql[:, 2 * k : 2 * k + 2, :],
                        rhs=qw_tiles[k][:, :, n * NF : (n + 1) * NF],
                        start=(k == 0),
                        stop=(k == n_pairs - 1),
                        perf_mode=DR,
                    )
            row = q * MS
            for n in range(n_tiles):
                t1 = ostg.tile([P, NF], F32, name="t1")
                nc.vector.tensor_tensor(
                    t1[:], pss[n][:], bP[:, n * NF : (n + 1) * NF], op=ALU.add
                )
                ot = ostg.tile([P, NF], F32, name="ot")
                nc.scalar.activation(
                    ot[:], t1[:], act_fn, bias=zbias[:], scale=C_b[:]
                )
                nc.gpsimd.dma_start(
                    out_sh[row : row + P, n * NF : (n + 1) * NF], ot[:]
                )

    nc.compile()
    return nc


_CACHE = {}


def _get_program(d=D, f_shard=F // NCORES, t_total=T, n_cores=NCORES):
    key = (d, f_shard, t_total, n_cores)
    if key not in _CACHE:
        _CACHE[key] = build_program(d, f_shard, t_total, n_cores)
    return _CACHE[key]


def make_in_maps(x, w, bias, in_hist, k_hist, n_cores=NCORES):
    t_total = x.shape[0]
    d = x.shape[1]
    t_loc = t_total // n_cores
    f_shard = w.shape[1] // n_cores
    d_tiles = d // P
    n_pairs = d_tiles // 2
    n_chunks = t_total // MS

    # x^T fp32 token-slices for exact amax
    xt = np.ascontiguousarray(x.T, dtype=np.float32)  # [D, T]
    # chunk-major fp16 x^T: L[q, p, s, m] = x[q*MS + m, s*P + p]
    x16 = x.astype(np.float16).reshape(n_chunks, MS, d_tiles, P)
    xt16 = np.ascontiguousarray(x16.transpose(0, 3, 2, 1))  # [64, 128, 32, 128]

    ih = np.asarray(in_hist, np.float32).reshape(1, HL)
    kh = np.asarray(k_hist, np.float32).reshape(1, HL)
    in_maps = []
    for r in range(n_cores):
        wsh = np.ascontiguousarray(
            w[:, r * f_shard : (r + 1) * f_shard], dtype=np.float32
        )
        # pair-major fp16 W: w16[k, p, o, f] = w[(2k+o)*P + p, f]
        w16 = np.ascontiguousarray(
            wsh.astype(np.float16)
            .reshape(n_pairs, 2, P, f_shard)
            .transpose(0, 2, 1, 3)
        )
        in_maps.append(
            {
                "xt16": xt16,
                "x_loc": np.ascontiguousarray(
                    xt[:, r * t_loc : (r + 1) * t_loc]
                ),
                "w_shard": wsh,
                "w16": w16,
                "bias_shard": np.ascontiguousarray(
                    bias[r * f_shard : (r + 1) * f_shard], dtype=np.float32
                ).reshape(1, f_shard),
                "in_hist": ih,
                "k_hist": kh,
            }
        )
    return in_maps


def _install_ntff_shim():
    """Provide antenv.axon_hooks (absent in this image) so bass_utils can
    NTFF-profile under axon, wiring it to libaxon_pjrt's nrt profile API."""
    import sys as _sys
    import types

    if "antenv.axon_hooks" in _sys.modules:
        return
    mod = types.ModuleType("antenv.axon_hooks")
    _state = {"hook": None}
    mod.set_axon_ntff_profile_hook = lambda h: _state.__setitem__("hook", h)
    mod.get_axon_ntff_profile_hook = lambda: _state["hook"]
    _sys.modules["antenv.axon_hooks"] = mod
    import antenv

    antenv.axon_hooks = mod
    try:
        from trn_agent_boot.trn_boot import _ntff_profile_via_ctypes

        mod.set_axon_ntff_profile_hook(
            _ntff_profile_via_ctypes("/opt/axon/libaxon_pjrt.so")
        )
    except Exception as e:
        print(f"ntff shim: hook unavailable ({e}); tracing will degrade")


def run(inputs_dict, trace=False, **kw):
    """Run on the 8 NeuronCores; returns (full_output, BassKernelResults)."""
    if trace:
        _install_ntff_shim()
    x = np.asarray(inputs_dict["inputs"], np.float32).reshape(T, D)
    w = np.asarray(inputs_dict["kernel"], np.float32)
    bias = np.asarray(inputs_dict["bias"], np.float32)
    nc = _get_program()
    in_maps = make_in_maps(
        x, w, bias, inputs_dict["input_amax_history"], inputs_dict["kernel_amax_history"]
    )
    old_m = nc.m
    nc.m = get_hw_module(nc.m)
    try:
        res = run_bass_kernel_spmd(
            nc, in_maps, core_ids=list(range(NCORES)), trace=trace, **kw
        )
    finally:
        nc.m = old_m
    out = np.concatenate(
        [res.results[r]["out_shard"] for r in range(NCORES)], axis=1
    )
    return out.reshape(B, S, F).astype(np.float32), res


def kernel(**inputs):
    out, _ = run(inputs, trace=False)
    return out


# revision 9
# speedup vs baseline: 1.2971x; 1.0276x over previous
"""Trainium2 Bass kernel for fp8 quantize-dequantize DenseGeneral + gelu.

Computes: out = gelu(qdq_e4m3fn(x) @ qdq_e4m3fn(W) + round_bf16(bias))
with delayed-scaling fp8 quantization (scale = amax/448 over full tensor,
folded with the amax history), reproducing reference.py numerics.

Distribution (8 NeuronCores, tensor-parallel on F):
  - The host pre-transposes x to x^T and replicates it to every core's
    DRAM in fp16, laid out chunk-major ([64, 128, 32, 128]: 1 MB fully
    contiguous per 128-token chunk) so the matmul lhsT stream runs at
    DMA line rate. Each core computes out[:, its 2048 f-columns] for ALL
    tokens, quantizing x^T chunks on the fly. No AllGather, no PE
    transposes, no runtime-offset DMAs.
  - Amaxes are reduced from the fp16 copies (each core: a disjoint 1/8
    chunk-slice of x^T and its W shard); one 2-float AllReduce(max)
    produces both global amaxes. The first 6 W pairs are parked in SBUF
    during the amax pass so quantization starts the moment the
    AllReduce lands; the rest re-stream.
  - fp16 transport shifts ~0.3% of fp8 rounding decisions and the scale
    by <=2^-11 vs the fp32 reference (measured offline: rel err 1.2e-2
    vs the 2e-2 gate).

fp8 trick: TRN's float8e4 has max 240 (OCP e4m3fn has 448). We store q/2:
multiplying by a power of two preserves round-to-nearest decisions on the
3-bit-mantissa grid, so RNE(v/2) in TRN-fp8 == RNE(v)/2 in e4m3fn for all
|v| >= 2^-5 (below that, absolute error <= 2^-9 * scale - negligible).
The factor 4 is folded into the output scale C = 4 * s_x * s_w.
The matmul runs in fp8 DoubleRow mode (2 fp8 MACs/cell/cycle).
"""

import sys

sys.path.insert(0, "/opt/trn_rl_repo")

import numpy as np
from contextlib import ExitStack

import concourse.bass as bass
import concourse.mybir as mybir
import concourse.tile as tile
from concourse import bacc, bass_isa
from concourse.bass_utils import run_bass_kernel_spmd
from concourse.bass_interp import get_hw_module

F32 = mybir.dt.float32
F16 = mybir.dt.float16
BF16 = mybir.dt.bfloat16
FP8 = mybir.dt.float8e4
AX = mybir.AxisListType
ALU = mybir.AluOpType
ACTF = mybir.ActivationFunctionType
DR = mybir.MatmulPerfMode.DoubleRow

# Problem shapes (hardcoded per contract)
B, S, D, F = 4, 2048, 4096, 16384
T = B * S
NCORES = 8
HL = 16
E4M3_MAX = 448.0
P = 128
MS = 128                 # tokens per lhsT chunk
WPARK = 6                # W pairs parked in SBUF during the amax pass


def build_program(d, f_shard, t_total, n_cores, hl=HL, act_fn=ACTF.Gelu_apprx_tanh):
    """Build the SPMD per-core bass program. Same program on every core;
    per-core behavior differs only through the input shards."""
    d_tiles = d // P             # 32 contraction subtiles
    n_pairs = d_tiles // 2       # 16 DoubleRow k-pairs
    NF = 512                     # psum free dim
    n_tiles = f_shard // NF      # 4
    n_chunks = t_total // MS     # 64
    loc_chunks = n_chunks // n_cores  # 8 chunks in this core's amax slice

    nc = bacc.Bacc(
        "TRN2",
        target_bir_lowering=False,
        debug=False,
        num_devices=n_cores,
    )

    xt16 = nc.dram_tensor("xt16", [n_chunks, P, d_tiles, MS], F16, kind="ExternalInput")
    x16l = nc.dram_tensor("x16_loc", [loc_chunks, P, d_tiles, MS], F16, kind="ExternalInput")
    w16 = nc.dram_tensor("w16", [n_pairs, P, 2, f_shard], F16, kind="ExternalInput")
    b_sh = nc.dram_tensor("bias_shard", [1, f_shard], F32, kind="ExternalInput")
    ih = nc.dram_tensor("in_hist", [1, hl], F32, kind="ExternalInput")
    kh = nc.dram_tensor("k_hist", [1, hl], F32, kind="ExternalInput")
    out_sh = nc.dram_tensor("out_shard", [t_total, f_shard], F32, kind="ExternalOutput")

    rg = [list(range(n_cores))]
    shared = "Shared" if n_cores > 4 else "Local"

    with tile.TileContext(nc) as tc, ExitStack() as ctx:
        const = ctx.enter_context(tc.tile_pool(name="const", bufs=1))
        small = ctx.enter_context(tc.tile_pool(name="small", bufs=1))
        wpark = ctx.enter_context(tc.tile_pool(name="wpark", bufs=1))
        wsr = ctx.enter_context(tc.tile_pool(name="wsr", bufs=3))
        qwp = ctx.enter_context(tc.tile_pool(name="qw", bufs=1))
        lfp = ctx.enter_context(tc.tile_pool(name="lfp", bufs=3))
        qlp = ctx.enter_context(tc.tile_pool(name="qlp", bufs=3))
        ostg = ctx.enter_context(tc.tile_pool(name="ostg", bufs=3))
        psum = ctx.enter_context(tc.tile_pool(name="psum", bufs=8, space="PSUM"))
        dram = ctx.enter_context(tc.tile_pool(name="dram", bufs=1, space="DRAM"))

        # DMA-trigger queues for the bulk prologue streams.
        queues = [nc.sync, nc.scalar, nc.gpsimd]

        # ---- constants ----
        zbias = const.tile([P, 1], F32)
        nc.gpsimd.memset(zbias[:], 0.0)

        histx = small.tile([1, hl], F32)
        nc.gpsimd.dma_start(histx[:], ih[:])
        histw = small.tile([1, hl], F32)
        nc.gpsimd.dma_start(histw[:], kh[:])

        # ---- phase 1: local abs-max from the fp16 copies ----
        # 8 x-chunks (1 MB each) + 16 w-pairs (1 MB each); first WPARK w
        # pairs land in dedicated park tiles and skip the later re-read.
        xacc = small.tile([P, loc_chunks], F32)
        wacc = small.tile([P, n_pairs], F32)
        wp_tiles = [
            wpark.tile([P, 2, f_shard], F16, name=f"wp{k}") for k in range(WPARK)
        ]
        qi = 0
        for i in range(loc_chunks):
            xt = lfp.tile([P, d_tiles, MS], F16, name="lf")
            queues[qi % 3].dma_start(xt[:], x16l[i])
            qi += 1
            nc.vector.reduce_max(
                xacc[:, i : i + 1], xt[:], axis=AX.XY, apply_absolute_value=True
            )
        for k in range(n_pairs):
            if k < WPARK:
                wt = wp_tiles[k]
            else:
                wt = wsr.tile([P, 2, f_shard], F16, name="wtr")
            queues[qi % 3].dma_start(wt[:], w16[k])
            qi += 1
            nc.vector.reduce_max(
                wacc[:, k : k + 1], wt[:], axis=AX.XY, apply_absolute_value=True
            )

        # cross-partition reduce both, pack into [1,2], ONE AllReduce(max)
        ar_in = dram.tile([1, 2], F32, name="arin")
        ar_out = dram.tile([1, 2], F32, addr_space=shared, name="arout")

        def pack_amax(acc, col, sfx):
            mp = small.tile([P, 1], F32, name=f"mp_{sfx}")
            nc.vector.reduce_max(mp[:], acc[:], axis=AX.X)
            ma = small.tile([P, 1], F32, name=f"ma_{sfx}")
            nc.gpsimd.partition_all_reduce(ma[:], mp[:], 128, bass_isa.ReduceOp.max)
            nc.gpsimd.dma_start(ar_in[:, col : col + 1], ma[0:1, :])

        pack_amax(xacc, 0, "x")
        pack_amax(wacc, 1, "w")
        nc.gpsimd.collective_compute(
            "AllReduce",
            ALU.max,
            replica_groups=rg,
            ins=[ar_in[:].opt()],
            outs=[ar_out[:].opt()],
        )
        g = small.tile([1, 2], F32)
        nc.gpsimd.dma_start(g[:], ar_out[:])

        # ---- phase 2: scales ----
        # reference: hist' = [amax_now, hist[0:HL-1]]; amax = max(hist')
        #            sf = 448/amax ; s = 1/sf (dequant scale)
        # ours:      r_half = 0.5*sf (quant multiplier, half-scale trick)
        #            C = 4 * s_x * s_w (output scale)
        def scales(gm, hist, sfx):
            hmx = small.tile([1, 1], F32, name=f"hmx_{sfx}")
            nc.vector.reduce_max(hmx[:], hist[:, 0 : hl - 1], axis=AX.X)
            amax = small.tile([1, 1], F32, name=f"amax_{sfx}")
            nc.vector.tensor_tensor(amax[:], gm, hmx[:], op=ALU.max)
            ra = small.tile([1, 1], F32, name=f"ra_{sfx}")
            nc.vector.reciprocal(ra[:], amax[:])
            sf = small.tile([1, 1], F32, name=f"sf_{sfx}")
            nc.vector.tensor_scalar_mul(sf[:], ra[:], E4M3_MAX)
            s = small.tile([1, 1], F32, name=f"s_{sfx}")
            nc.vector.reciprocal(s[:], sf[:])
            rh = small.tile([1, 1], F32, name=f"rh_{sfx}")
            nc.vector.tensor_scalar_mul(rh[:], sf[:], 0.5)
            return s, rh

        s_x, rh_x = scales(g[:, 0:1], histx, "x")
        s_w, rh_w = scales(g[:, 1:2], histw, "w")

        Cs = small.tile([1, 1], F32)
        nc.vector.tensor_tensor(Cs[:], s_x[:], s_w[:], op=ALU.mult)
        nc.vector.tensor_scalar_mul(Cs[:], Cs[:], 4.0)
        rC = small.tile([1, 1], F32)
        nc.vector.reciprocal(rC[:], Cs[:])

        rhx_b = small.tile([P, 1], F32)
        nc.gpsimd.partition_broadcast(rhx_b[:], rh_x[:])
        rhw_b = small.tile([P, 1], F32)
        nc.gpsimd.partition_broadcast(rhw_b[:], rh_w[:])
        C_b = small.tile([P, 1], F32)
        nc.gpsimd.partition_broadcast(C_b[:], Cs[:])

        # bias: fp32 -> bf16 -> fp32, then pre-divide by C, broadcast to 128 parts
        btmp = small.tile([1, f_shard], F32)
        nc.gpsimd.dma_start(btmp[:], b_sh[:])
        bbf = small.tile([1, f_shard], BF16)
        nc.vector.tensor_copy(bbf[:], btmp[:])
        nc.vector.tensor_copy(btmp[:], bbf[:])
        nc.vector.tensor_scalar_mul(btmp[:], btmp[:], rC[:])
        bP = small.tile([P, f_shard], F32)
        nc.gpsimd.partition_broadcast(bP[:], btmp[:])

        # lhsT chunk loader: 1 MB contiguous fp16 DMA + quantize on ScalarE
        def load_chunk(q):
            lf = lfp.tile([P, d_tiles, MS], F16, name="lf")
            nc.sync.dma_start(lf[:], xt16[q])
            ql = qlp.tile([P, d_tiles, MS], FP8, name="ql")
            nc.scalar.mul(ql[:], lf[:], rhx_b[:])
            return ql

        # preload chunks 0/1 ahead of the w-quant muls in the ScalarE FIFO
        # so the first matmuls fire right after the AllReduce
        pre = {q: load_chunk(q) for q in (0, 1)}

        # ---- phase 3: quantize w (parked pairs instantly, rest re-stream) ----
        # Quant muls alternate ScalarE/VectorE so the matmul k-loop chases
        # at ~1.5us/pair instead of 3us.
        qw_tiles = [
            qwp.tile([P, 2, f_shard], FP8, name=f"qwt{k}") for k in range(n_pairs)
        ]
        for k in range(n_pairs):
            if k < WPARK:
                wt = wp_tiles[k]
            else:
                wt = wsr.tile([P, 2, f_shard], F16, name="wtr")
                queues[(k % 2) + 1].dma_start(wt[:], w16[k])
            if k % 2 == 0:
                nc.scalar.mul(qw_tiles[k][:], wt[:], rhw_b[:])
            else:
                nc.vector.tensor_scalar_mul(qw_tiles[k][:], wt[:], rhw_b[:])

        # ---- phase 4: matmul + epilogue ----
        # out[tok, f] = gelu(C * (sum_d qxT[d, tok] * qw[d, f] + bias/C))
        for q in range(n_chunks):
            ql = pre.pop(q) if q in pre else load_chunk(q)
            pss = [
                psum.tile([P, NF], F32, tag="ps", name=f"mmps{n}")
                for n in range(n_tiles)
            ]
            for k in range(n_pairs):
                for n in range(n_tiles):
                    nc.tensor.matmul(
                        pss[n][:],
                        lhsT=ql[:, 2 * k : 2 * k + 2, :],
                        rhs=qw_tiles[k][:, :, n * NF : (n + 1) * NF],
                        start=(k == 0),
                        stop=(k == n_pairs - 1),
                        perf_mode=DR,
                    )
            row = q * MS
            for n in range(n_tiles):
                t1 = ostg.tile([P, NF], F32, name="t1")
                nc.vector.tensor_tensor(
                    t1[:], pss[n][:], bP[:, n * NF : (n + 1) * NF], op=ALU.add
                )
                ot = ostg.tile([P, NF], F32, name="ot")
                nc.scalar.activation(
                    ot[:], t1[:], act_fn, bias=zbias[:], scale=C_b[:]
                )
                nc.gpsimd.dma_start(
                    out_sh[row : row + P, n * NF : (n + 1) * NF], ot[:]
                )

    nc.compile()
    return nc


_CACHE = {}


def _get_program(d=D, f_shard=F // NCORES, t_total=T, n_cores=NCORES):
    key = (d, f_shard, t_total, n_cores)
    if key not in _CACHE:
        _CACHE[key] = build_program(d, f_shard, t_total, n_cores)
    return _CACHE[key]


def make_in_maps(x, w, bias, in_hist, k_hist, n_cores=NCORES):
    t_total = x.shape[0]
    d = x.shape[1]
    f_shard = w.shape[1] // n_cores
    d_tiles = d // P
    n_pairs = d_tiles // 2
    n_chunks = t_total // MS
    loc_chunks = n_chunks // n_cores

    # chunk-major fp16 x^T: L[q, p, s, m] = x[q*MS + m, s*P + p]
    x16 = x.astype(np.float16).reshape(n_chunks, MS, d_tiles, P)
    xt16 = np.ascontiguousarray(x16.transpose(0, 3, 2, 1))  # [64, 128, 32, 128]

    ih = np.asarray(in_hist, np.float32).reshape(1, HL)
    kh = np.asarray(k_hist, np.float32).reshape(1, HL)
    in_maps = []
    for r in range(n_cores):
        # pair-major fp16 W: w16[k, p, o, f] = w[(2k+o)*P + p, f]
        w16 = np.ascontiguousarray(
            w[:, r * f_shard : (r + 1) * f_shard]
            .astype(np.float16)
            .reshape(n_pairs, 2, P, f_shard)
            .transpose(0, 2, 1, 3)
        )
        in_maps.append(
            {
                "xt16": xt16,
                "x16_loc": np.ascontiguousarray(
                    xt16[r * loc_chunks : (r + 1) * loc_chunks]
                ),
                "w16": w16,
                "bias_shard": np.ascontiguousarray(
                    bias[r * f_shard : (r + 1) * f_shard], dtype=np.float32
                ).reshape(1, f_shard),
                "in_hist": ih,
                "k_hist": kh,
            }
        )
    return in_maps


def _install_ntff_shim():
    """Provide antenv.axon_hooks (absent in this image) so bass_utils can
    NTFF-profile under axon, wiring it to libaxon_pjrt's nrt profile API."""
    import sys as _sys
    import types

    if "antenv.axon_hooks" in _sys.modules:
        return
    mod = types.ModuleType("antenv.axon_hooks")
    _state = {"hook": None}
    mod.set_axon_ntff_profile_hook = lambda h: _state.__setitem__("hook", h)
    mod.get_axon_ntff_profile_hook = lambda: _state["hook"]
    _sys.modules["antenv.axon_hooks"] = mod
    import antenv

    antenv.axon_hooks = mod
    try:
        from trn_agent_boot.trn_boot import _ntff_profile_via_ctypes

        mod.set_axon_ntff_profile_hook(
            _ntff_profile_via_ctypes("/opt/axon/libaxon_pjrt.so")
        )
    except Exception as e:
        print(f"ntff shim: hook unavailable ({e}); tracing will degrade")


def run(inputs_dict, trace=False, **kw):
    """Run on the 8 NeuronCores; returns (full_output, BassKernelResults)."""
    if trace:
        _install_ntff_shim()
    x = np.asarray(inputs_dict["inputs"], np.float32).reshape(T, D)
    w = np.asarray(inputs_dict["kernel"], np.float32)
    bias = np.asarray(inputs_dict["bias"], np.float32)
    nc = _get_program()
    in_maps = make_in_maps(
        x, w, bias, inputs_dict["input_amax_history"], inputs_dict["kernel_amax_history"]
    )
    old_m = nc.m
    nc.m = get_hw_module(nc.m)
    try:
        res = run_bass_kernel_spmd(
            nc, in_maps, core_ids=list(range(NCORES)), trace=trace, **kw
        )
    finally:
        nc.m = old_m
    out = np.concatenate(
        [res.results[r]["out_shard"] for r in range(NCORES)], axis=1
    )
    return out.reshape(B, S, F).astype(np.float32), res


def kernel(**inputs):
    out, _ = run(inputs, trace=False)
    return out


# revision 25
# speedup vs baseline: 1.3238x; 1.0205x over previous
"""Trainium2 Bass kernel for fp8 quantize-dequantize DenseGeneral + gelu.

Computes: out = gelu(qdq_e4m3fn(x) @ qdq_e4m3fn(W) + round_bf16(bias))
with delayed-scaling fp8 quantization (scale = amax/448 over full tensor,
folded with the amax history), reproducing reference.py numerics.

Distribution (8 NeuronCores, tensor-parallel on F):
  - The host pre-transposes x to x^T and replicates it to every core's
    DRAM in fp16, laid out chunk-major ([64, 128, 32, 128]: 1 MB fully
    contiguous per 128-token chunk) so the matmul lhsT stream runs at
    DMA line rate. Each core computes out[:, its 2048 f-columns] for ALL
    tokens, quantizing x^T chunks on the fly. No AllGather, no PE
    transposes, no runtime-offset DMAs.
  - Amaxes are reduced from the fp16 copies (each core: a disjoint 1/8
    chunk-slice of x^T and its W shard); one 2-float AllReduce(max)
    produces both global amaxes. The first 6 W pairs are parked in SBUF
    during the amax pass so quantization starts the moment the
    AllReduce lands; the rest re-stream.
  - fp16 transport shifts ~0.3% of fp8 rounding decisions and the scale
    by <=2^-11 vs the fp32 reference (measured offline: rel err 1.2e-2
    vs the 2e-2 gate).

fp8 trick: TRN's float8e4 has max 240 (OCP e4m3fn has 448). We store q/2:
multiplying by a power of two preserves round-to-nearest decisions on the
3-bit-mantissa grid, so RNE(v/2) in TRN-fp8 == RNE(v)/2 in e4m3fn for all
|v| >= 2^-5 (below that, absolute error <= 2^-9 * scale - negligible).
The factor 4 is folded into the output scale C = 4 * s_x * s_w.
The matmul runs in fp8 DoubleRow mode (2 fp8 MACs/cell/cycle).
"""

import sys

sys.path.insert(0, "/opt/trn_rl_repo")

import numpy as np
from contextlib import ExitStack

import concourse.bass as bass
import concourse.mybir as mybir
import concourse.tile as tile
from concourse import bacc, bass_isa
from concourse.bass_utils import run_bass_kernel_spmd
from concourse.bass_interp import get_hw_module

F32 = mybir.dt.float32
F16 = mybir.dt.float16
BF16 = mybir.dt.bfloat16
FP8 = mybir.dt.float8e4
AX = mybir.AxisListType
ALU = mybir.AluOpType
ACTF = mybir.ActivationFunctionType
DR = mybir.MatmulPerfMode.DoubleRow

# Problem shapes (hardcoded per contract)
B, S, D, F = 4, 2048, 4096, 16384
T = B * S
NCORES = 8
HL = 16
E4M3_MAX = 448.0
P = 128
MS = 128                 # tokens per lhsT chunk
WPARK = 6                # W pairs parked in SBUF during the amax pass


def build_program(d, f_shard, t_total, n_cores, hl=HL, act_fn=ACTF.Gelu_apprx_tanh):
    """Build the SPMD per-core bass program. Same program on every core;
    per-core behavior differs only through the input shards."""
    d_tiles = d // P             # 32 contraction subtiles
    n_pairs = d_tiles // 2       # 16 DoubleRow k-pairs
    NF = 512                     # psum free dim
    n_tiles = f_shard // NF      # 4
    n_chunks = t_total // MS     # 64
    loc_chunks = n_chunks // n_cores  # 8 chunks in this core's amax slice

    nc = bacc.Bacc(
        "TRN2",
        target_bir_lowering=False,
        debug=False,
        num_devices=n_cores,
    )

    xt16 = nc.dram_tensor("xt16", [n_chunks, P, d_tiles, MS], F16, kind="ExternalInput")
    x16l = nc.dram_tensor("x16_loc", [loc_chunks, P, d_tiles, MS], F16, kind="ExternalInput")
    w16 = nc.dram_tensor("w16", [n_pairs, P, 2, f_shard], F16, kind="ExternalInput")
    b_sh = nc.dram_tensor("bias_shard", [1, f_shard], F32, kind="ExternalInput")
    ih = nc.dram_tensor("in_hist", [1, hl], F32, kind="ExternalInput")
    kh = nc.dram_tensor("k_hist", [1, hl], F32, kind="ExternalInput")
    out_sh = nc.dram_tensor("out_shard", [t_total, f_shard], F32, kind="ExternalOutput")

    rg = [list(range(n_cores))]
    shared = "Shared" if n_cores > 4 else "Local"

    with tile.TileContext(nc) as tc, ExitStack() as ctx:
        const = ctx.enter_context(tc.tile_pool(name="const", bufs=1))
        small = ctx.enter_context(tc.tile_pool(name="small", bufs=1))
        wpark = ctx.enter_context(tc.tile_pool(name="wpark", bufs=1))
        wsr = ctx.enter_context(tc.tile_pool(name="wsr", bufs=3))
        qwp = ctx.enter_context(tc.tile_pool(name="qw", bufs=1))
        lfp = ctx.enter_context(tc.tile_pool(name="lfp", bufs=3))
        qlp = ctx.enter_context(tc.tile_pool(name="qlp", bufs=3))
        ostg = ctx.enter_context(tc.tile_pool(name="ostg", bufs=3))
        psum = ctx.enter_context(tc.tile_pool(name="psum", bufs=8, space="PSUM"))
        dram = ctx.enter_context(tc.tile_pool(name="dram", bufs=1, space="DRAM"))

        # DMA-trigger queues for the bulk prologue streams (HWDGE only;
        # the gpsimd engine is kept clear to run half the amax reduces).
        queues = [nc.sync, nc.scalar]

        # ---- constants ----
        zbias = const.tile([P, 1], F32)
        nc.gpsimd.memset(zbias[:], 0.0)

        histx = small.tile([1, hl], F32)
        nc.gpsimd.dma_start(histx[:], ih[:])
        histw = small.tile([1, hl], F32)
        nc.gpsimd.dma_start(histw[:], kh[:])

        # ---- phase 1: local abs-max from the fp16 copies ----
        # 8 x-chunks (1 MB each) + 16 w-pairs (1 MB each), DMAs
        # interleaved across both HWDGE queues. All reduces on the DVE
        # (tensor_reduce only has a 1x uop: ~4.4us/tile, ~106us total —
        # the binding prologue resource, overlapped with the DMA stream).
        # First WPARK w pairs land in park tiles and skip the re-read.
        xacc = small.tile([P, loc_chunks], F32)
        wacc = small.tile([P, n_pairs], F32)
        wp_tiles = [
            wpark.tile([P, 2, f_shard], F16, name=f"wp{k}") for k in range(WPARK)
        ]

        qi = 0
        for step in range(n_pairs):
            if step < loc_chunks:
                xt = lfp.tile([P, d_tiles, MS], F16, name="lf")
                queues[qi % 2].dma_start(xt[:], x16l[step])
                qi += 1
                nc.vector.reduce_max(
                    xacc[:, step : step + 1], xt[:], axis=AX.XY,
                    apply_absolute_value=True,
                )
            k = step
            if k < WPARK:
                wt = wp_tiles[k]
            else:
                wt = wsr.tile([P, 2, f_shard], F16, name="wtr")
            queues[qi % 2].dma_start(wt[:], w16[k])
            qi += 1
            nc.vector.reduce_max(
                wacc[:, k : k + 1], wt[:], axis=AX.XY,
                apply_absolute_value=True,
            )

        # cross-partition reduce both, pack into [1,2], ONE AllReduce(max)
        ar_in = dram.tile([1, 2], F32, name="arin")
        ar_out = dram.tile([1, 2], F32, addr_space=shared, name="arout")

        mp = small.tile([P, 2], F32)
        nc.vector.reduce_max(mp[:, 0:1], xacc[:], axis=AX.X)
        nc.vector.reduce_max(mp[:, 1:2], wacc[:], axis=AX.X)
        ma = small.tile([P, 2], F32)
        nc.gpsimd.partition_all_reduce(ma[:], mp[:], 128, bass_isa.ReduceOp.max)
        nc.gpsimd.dma_start(ar_in[:], ma[0:1, :])
        nc.gpsimd.collective_compute(
            "AllReduce",
            ALU.max,
            replica_groups=rg,
            ins=[ar_in[:].opt()],
            outs=[ar_out[:].opt()],
        )
        g = small.tile([1, 2], F32)
        nc.gpsimd.dma_start(g[:], ar_out[:])

        # ---- phase 2: scales ----
        # reference: hist' = [amax_now, hist[0:HL-1]]; amax = max(hist')
        #            sf = 448/amax ; s = 1/sf (dequant scale)
        # ours:      r_half = 0.5*sf (quant multiplier, half-scale trick)
        #            C = 4 * s_x * s_w (output scale)
        def scales(gm, hist, sfx):
            hmx = small.tile([1, 1], F32, name=f"hmx_{sfx}")
            nc.vector.reduce_max(hmx[:], hist[:, 0 : hl - 1], axis=AX.X)
            amax = small.tile([1, 1], F32, name=f"amax_{sfx}")
            nc.vector.tensor_tensor(amax[:], gm, hmx[:], op=ALU.max)
            ra = small.tile([1, 1], F32, name=f"ra_{sfx}")
            nc.vector.reciprocal(ra[:], amax[:])
            sf = small.tile([1, 1], F32, name=f"sf_{sfx}")
            nc.vector.tensor_scalar_mul(sf[:], ra[:], E4M3_MAX)
            s = small.tile([1, 1], F32, name=f"s_{sfx}")
            nc.vector.reciprocal(s[:], sf[:])
            rh = small.tile([1, 1], F32, name=f"rh_{sfx}")
            nc.vector.tensor_scalar_mul(rh[:], sf[:], 0.5)
            return s, rh

        s_x, rh_x = scales(g[:, 0:1], histx, "x")
        s_w, rh_w = scales(g[:, 1:2], histw, "w")

        Cs = small.tile([1, 1], F32)
        nc.vector.tensor_tensor(Cs[:], s_x[:], s_w[:], op=ALU.mult)
        nc.vector.tensor_scalar_mul(Cs[:], Cs[:], 4.0)
        rC = small.tile([1, 1], F32)
        nc.vector.reciprocal(rC[:], Cs[:])

        rhx_b = small.tile([P, 1], F32)
        nc.gpsimd.partition_broadcast(rhx_b[:], rh_x[:])
        rhw_b = small.tile([P, 1], F32)
        nc.gpsimd.partition_broadcast(rhw_b[:], rh_w[:])
        C_b = small.tile([P, 1], F32)
        nc.gpsimd.partition_broadcast(C_b[:], Cs[:])

        # bias: fp32 -> bf16 -> fp32, then pre-divide by C, broadcast to 128 parts
        btmp = small.tile([1, f_shard], F32)
        nc.gpsimd.dma_start(btmp[:], b_sh[:])
        bbf = small.tile([1, f_shard], BF16)
        nc.vector.tensor_copy(bbf[:], btmp[:])
        nc.vector.tensor_copy(btmp[:], bbf[:])
        nc.vector.tensor_scalar_mul(btmp[:], btmp[:], rC[:])
        bP = small.tile([P, f_shard], F32)
        nc.gpsimd.partition_broadcast(bP[:], btmp[:])

        # lhsT chunk loader: 1 MB contiguous fp16 DMA + quantize on ScalarE
        def load_chunk(q):
            lf = lfp.tile([P, d_tiles, MS], F16, name="lf")
            nc.sync.dma_start(lf[:], xt16[q])
            ql = qlp.tile([P, d_tiles, MS], FP8, name="ql")
            nc.scalar.mul(ql[:], lf[:], rhx_b[:])
            return ql

        # preload chunks 0/1 ahead of the w-quant muls in the ScalarE FIFO
        # so the first matmuls fire right after the AllReduce
        pre = {q: load_chunk(q) for q in (0, 1)}

        # ---- phase 3: quantize w (parked pairs instantly, rest re-stream) ----
        # Quant muls alternate ScalarE/VectorE so the matmul k-loop chases
        # at ~1.5us/pair instead of 3us.
        qw_tiles = [
            qwp.tile([P, 2, f_shard], FP8, name=f"qwt{k}") for k in range(n_pairs)
        ]
        for k in range(n_pairs):
            if k < WPARK:
                wt = wp_tiles[k]
            else:
                wt = wsr.tile([P, 2, f_shard], F16, name="wtr")
                (nc.scalar if k % 2 == 0 else nc.gpsimd).dma_start(wt[:], w16[k])
            if k % 2 == 0:
                nc.scalar.mul(qw_tiles[k][:], wt[:], rhw_b[:])
            else:
                nc.vector.tensor_scalar_mul(qw_tiles[k][:], wt[:], rhw_b[:])

        # ---- phase 4: matmul + epilogue ----
        # out[tok, f] = gelu(C * (sum_d qxT[d, tok] * qw[d, f] + bias/C))
        for q in range(n_chunks):
            ql = pre.pop(q) if q in pre else load_chunk(q)
            pss = [
                psum.tile([P, NF], F32, tag="ps", name=f"mmps{n}")
                for n in range(n_tiles)
            ]
            for k in range(n_pairs):
                for n in range(n_tiles):
                    nc.tensor.matmul(
                        pss[n][:],
                        lhsT=ql[:, 2 * k : 2 * k + 2, :],
                        rhs=qw_tiles[k][:, :, n * NF : (n + 1) * NF],
                        start=(k == 0),
                        stop=(k == n_pairs - 1),
                        perf_mode=DR,
                    )
            row = q * MS
            for n in range(n_tiles):
                t1 = ostg.tile([P, NF], F32, name="t1")
                nc.vector.tensor_tensor(
                    t1[:], pss[n][:], bP[:, n * NF : (n + 1) * NF], op=ALU.add
                )
                ot = ostg.tile([P, NF], F32, name="ot")
                nc.scalar.activation(
                    ot[:], t1[:], act_fn, bias=zbias[:], scale=C_b[:]
                )
                oq = nc.gpsimd if n % 2 == 0 else nc.sync
                oq.dma_start(
                    out_sh[row : row + P, n * NF : (n + 1) * NF], ot[:]
                )

    nc.compile()
    return nc


_CACHE = {}


def _get_program(d=D, f_shard=F // NCORES, t_total=T, n_cores=NCORES):
    key = (d, f_shard, t_total, n_cores)
    if key not in _CACHE:
        _CACHE[key] = build_program(d, f_shard, t_total, n_cores)
    return _CACHE[key]


def make_in_maps(x, w, bias, in_hist, k_hist, n_cores=NCORES):
    t_total = x.shape[0]
    d = x.shape[1]
    f_shard = w.shape[1] // n_cores
    d_tiles = d // P
    n_pairs = d_tiles // 2
    n_chunks = t_total // MS
    loc_chunks = n_chunks // n_cores

    # chunk-major fp16 x^T: L[q, p, s, m] = x[q*MS + m, s*P + p]
    x16 = x.astype(np.float16).reshape(n_chunks, MS, d_tiles, P)
    xt16 = np.ascontiguousarray(x16.transpose(0, 3, 2, 1))  # [64, 128, 32, 128]

    ih = np.asarray(in_hist, np.float32).reshape(1, HL)
    kh = np.asarray(k_hist, np.float32).reshape(1, HL)
    in_maps = []
    for r in range(n_cores):
        # pair-major fp16 W: w16[k, p, o, f] = w[(2k+o)*P + p, f]
        w16 = np.ascontiguousarray(
            w[:, r * f_shard : (r + 1) * f_shard]
            .astype(np.float16)
            .reshape(n_pairs, 2, P, f_shard)
            .transpose(0, 2, 1, 3)
        )
        in_maps.append(
            {
                "xt16": xt16,
                "x16_loc": np.ascontiguousarray(
                    xt16[r * loc_chunks : (r + 1) * loc_chunks]
                ),
                "w16": w16,
                "bias_shard": np.ascontiguousarray(
                    bias[r * f_shard : (r + 1) * f_shard], dtype=np.float32
                ).reshape(1, f_shard),
                "in_hist": ih,
                "k_hist": kh,
            }
        )
    return in_maps


def _install_ntff_shim():
    """Provide antenv.axon_hooks (absent in this image) so bass_utils can
    NTFF-profile under axon, wiring it to libaxon_pjrt's nrt profile API."""
    import sys as _sys
    import types

    if "antenv.axon_hooks" in _sys.modules:
        return
    mod = types.ModuleType("antenv.axon_hooks")
    _state = {"hook": None}
    mod.set_axon_ntff_profile_hook = lambda h: _state.__setitem__("hook", h)
    mod.get_axon_ntff_profile_hook = lambda: _state["hook"]
    _sys.modules["antenv.axon_hooks"] = mod
    import antenv

    antenv.axon_hooks = mod
    try:
        from trn_agent_boot.trn_boot import _ntff_profile_via_ctypes

        mod.set_axon_ntff_profile_hook(
            _ntff_profile_via_ctypes("/opt/axon/libaxon_pjrt.so")
        )
    except Exception as e:
        print(f"ntff shim: hook unavailable ({e}); tracing will degrade")


def run(inputs_dict, trace=False, **kw):
    """Run on the 8 NeuronCores; returns (full_output, BassKernelResults)."""
    if trace:
        _install_ntff_shim()
    x = np.asarray(inputs_dict["inputs"], np.float32).reshape(T, D)
    w = np.asarray(inputs_dict["kernel"], np.float32)
    bias = np.asarray(inputs_dict["bias"], np.float32)
    nc = _get_program()
    in_maps = make_in_maps(
        x, w, bias, inputs_dict["input_amax_history"], inputs_dict["kernel_amax_history"]
    )
    old_m = nc.m
    nc.m = get_hw_module(nc.m)
    try:
        res = run_bass_kernel_spmd(
            nc, in_maps, core_ids=list(range(NCORES)), trace=trace, **kw
        )
    finally:
        nc.m = old_m
    out = np.concatenate(
        [res.results[r]["out_shard"] for r in range(NCORES)], axis=1
    )
    return out.reshape(B, S, F).astype(np.float32), res


def kernel(**inputs):
    out, _ = run(inputs, trace=False)
    return out
